# revision 8
# baseline (speedup 1.0000x reference)
"""Trainium2 Bass kernel for nn_LBLResNetBiLm (v2: bf16 matmuls + diag-matmul LN apply).

Computation (see reference): pad sequence with learned boundary vectors,
take 9-tap left/right weighted window sums over the sequence, then run 3
residual pre-LN FFN layers per branch; emit every layer's concat(left,right).

Sharding: data-parallel over batch, 2 batch elements per core x 8 cores.

Per-core design (tokens on partitions, d on free dim; residual x in fp32):
- window conv = band-matrix matmul over host-built Toeplitz bands (bf16)
- LN stats: bn_stats per (subtile, branch); even/odd halves merged with
  batched [128,8] DVE ops; LN gamma/beta folded into W1/b1 on host
- LN apply + transpose fused into PE: zT = xb^T @ diag(rstd) (diag built
  by gpsimd from a const identity), mean correction folded into mm1 as a
  K=1 matmul with lhsT = -0.5*colsum(W1') and rhs = (mean_e+mean_o)*rstd
  row (moved to free-dim layout via a small PE transpose + sbuf DMA)
- mm1: W1 chunks stationary, N=512; relu+b1 on ACT -> hs bf16
- mm2: hs chunks stationary, W2 moving -> y psum lands in [token, d]
  layout; b2 via K=1 ones matmul only if b2 != 0
- resid: DVE tensor_tensor add (fp32, exact residual stream)
"""
import sys

sys.path.insert(0, "/opt/trn_rl_repo")

from contextlib import ExitStack

import ml_dtypes
import numpy as np

import concourse.bass as bass
import concourse.tile as tile
from concourse import bacc, mybir
from concourse.bass_utils import run_bass_kernel_spmd
from concourse.masks import make_identity

B, S, D, W, L = 16, 2048, 256, 8, 3
NCORES = 8
BPC = B // NCORES            # batches per core
SUB = 128                    # tokens per subtile (partition dim)
SPT = 4                      # subtiles per supertile
ST = SUB * SPT               # tokens per supertile
NST = BPC * S // ST          # supertiles per core
F32 = mybir.dt.float32
BF16 = mybir.dt.bfloat16
EPS = 1e-6

_CACHE = {}


def _build_program(has_b1, has_b2):
    nc = bacc.Bacc("TRN2", target_bir_lowering=False, debug=False, num_devices=NCORES)

    xpad_d = nc.dram_tensor("xpad", [BPC, S + 2 * W, D], BF16, kind="ExternalInput").ap()
    # w1: [128(din-in-chunk), (l br k m) * 128(dout-in-chunk)]
    w1_d = nc.dram_tensor("w1", [128, L * 8 * 128], BF16, kind="ExternalInput").ap()
    # w2: [128(dout-in-chunk), (l br m) * 256(d2)]
    w2_d = nc.dram_tensor("w2", [128, L * 4 * 256], BF16, kind="ExternalInput").ap()
    # s1: [1, (l br m) * 128] = -0.5 * colsum of folded W1
    s1_d = nc.dram_tensor("s1", [1, L * 4 * 128], BF16, kind="ExternalInput").ap()
    b1_d = nc.dram_tensor("b1", [128, L * 2 * 2], F32, kind="ExternalInput").ap()
    b2_d = nc.dram_tensor("b2", [1, L * 2 * 256], BF16, kind="ExternalInput").ap()
    banda_d = nc.dram_tensor("banda", [128, 2 * 128], BF16, kind="ExternalInput").ap()
    bandb_d = nc.dram_tensor("bandb", [2 * W, 2 * 128], BF16, kind="ExternalInput").ap()
    out_d = nc.dram_tensor("out_all", [L, BPC, S, 2 * D], F32, kind="ExternalOutput").ap()

    with tile.TileContext(nc) as tc, ExitStack() as ctx:
        const = ctx.enter_context(tc.tile_pool(name="const", bufs=1))
        pads = ctx.enter_context(tc.tile_pool(name="pads", bufs=4))
        xs = ctx.enter_context(tc.tile_pool(name="xs", bufs=16))
        xbs = ctx.enter_context(tc.tile_pool(name="xbs", bufs=10))
        diags = ctx.enter_context(tc.tile_pool(name="diags", bufs=10))
        zts = ctx.enter_context(tc.tile_pool(name="zts", bufs=2))
        hss = ctx.enter_context(tc.tile_pool(name="hss", bufs=2))
        stat = ctx.enter_context(tc.tile_pool(name="stat", bufs=4))
        rows = ctx.enter_context(tc.tile_pool(name="rows", bufs=4))
        ps_a = ctx.enter_context(tc.tile_pool(name="ps_a", bufs=2, space="PSUM"))
        ps_h = ctx.enter_context(tc.tile_pool(name="ps_h", bufs=1, space="PSUM"))
        ps_y = ctx.enter_context(tc.tile_pool(name="ps_y", bufs=2, space="PSUM"))
        ps_r = ctx.enter_context(tc.tile_pool(name="ps_r", bufs=1, space="PSUM"))

        # ---- constants
        w1_sb = const.tile([128, L, 2, 2, 2, 128], BF16)
        nc.sync.dma_start(w1_sb[:], w1_d[:].rearrange("p (l b k m q) -> p l b k m q", l=L, b=2, k=2, m=2))
        w2_sb = const.tile([128, L, 2, 2, 256], BF16)
        nc.sync.dma_start(w2_sb[:], w2_d[:].rearrange("p (l b m q) -> p l b m q", l=L, b=2, m=2))
        s1_sb = const.tile([1, L, 2, 2, 128], BF16)
        nc.sync.dma_start(s1_sb[:], s1_d[:].rearrange("p (l b m q) -> p l b m q", l=L, b=2, m=2))
        b1_sb = const.tile([128, L, 2, 2], F32)
        nc.sync.dma_start(b1_sb[:], b1_d[:].rearrange("p (l b m) -> p l b m", l=L, b=2))
        b2_sb = None
        ones_sb = None
        if has_b2:
            b2_sb = const.tile([1, L, 2, 256], BF16)
            nc.sync.dma_start(b2_sb[:], b2_d[:].rearrange("p (l b q) -> p l b q", l=L, b=2))
            ones_f = const.tile([1, 128], F32)
            nc.vector.memset(ones_f[:], 1.0)
            ones_sb = const.tile([1, 128], BF16)
            nc.vector.tensor_copy(ones_sb[:], ones_f[:])
        banda_sb = const.tile([128, 2, 128], BF16)
        nc.sync.dma_start(banda_sb[:], banda_d[:].rearrange("p (b q) -> p b q", b=2))
        bandb_sb = const.tile([2 * W, 2, 128], BF16)
        nc.sync.dma_start(bandb_sb[:], bandb_d[:].rearrange("p (b q) -> p b q", b=2))

        ident_f = const.tile([128, 128], F32)
        make_identity(nc, ident_f[:])
        ident_bf = const.tile([128, 128], BF16)
        nc.vector.tensor_copy(ident_bf[:], ident_f[:])
        eps_t = const.tile([128, 1], F32)
        nc.vector.memset(eps_t[:], EPS)
        zero_t = const.tile([128, 1], F32)
        nc.vector.memset(zero_t[:], 0.0)

        # ---- main loop
        for st in range(NST):
            b = (st * ST) // S
            t0 = (st * ST) % S

            # conv: x0[t, (br,d)] per subtile (bf16 matmul, fp32 psum -> fp32 x)
            x_cur = []
            for sub in range(SPT):
                ts = t0 + sub * SUB
                pad_a = pads.tile([128, D], BF16, tag="pad_a")
                nc.sync.dma_start(pad_a[:], xpad_d[b, ts : ts + 128, :])
                pad_b = pads.tile([2 * W, D], BF16, tag="pad_b")
                nc.sync.dma_start(pad_b[:], xpad_d[b, ts + 128 : ts + 128 + 2 * W, :])

                cps = ps_a.tile([128, 2 * D], F32, tag="ps_a")
                for br in range(2):
                    nc.tensor.matmul(
                        cps[:, br * D : (br + 1) * D],
                        banda_sb[:, br, :], pad_a[:], start=True, stop=False,
                    )
                    nc.tensor.matmul(
                        cps[:, br * D : (br + 1) * D],
                        bandb_sb[:, br, :], pad_b[:], start=False, stop=True,
                    )
                x_t = xs.tile([128, 2, D], F32, tag="x")
                nc.scalar.copy(x_t[:], cps[:].rearrange("p (b q) -> p b q", b=2))
                x_cur.append(x_t)

            for l in range(L):
                # xb cast (ACT) + bn stats (DVE)
                xb_all = []
                stats = stat.tile([128, SPT, 2, 6], F32, tag="bnst")
                for sub in range(SPT):
                    xb_t = xbs.tile([128, 2, D], BF16, tag="xb")
                    nc.scalar.copy(xb_t[:], x_cur[sub][:])
                    xb_all.append(xb_t)
                    for br in range(2):
                        nc.vector.bn_stats(stats[:, sub, br, :], x_cur[sub][:, br, :])

                # batched stats finalize over [128, 8]  (cols = (sub, br))
                sv = stats[:].rearrange("p s b q -> p (s b) q")
                me, mo = sv[:, :, 1], sv[:, :, 4]
                m2e, m2o = sv[:, :, 2], sv[:, :, 5]
                dm = stat.tile([128, SPT * 2], F32, tag="dm")
                nc.vector.tensor_tensor(out=dm[:], in0=me, in1=mo, op=mybir.AluOpType.subtract)
                dm2 = stat.tile([128, SPT * 2], F32, tag="dm2")
                nc.vector.tensor_tensor(out=dm2[:], in0=dm[:], in1=dm[:], op=mybir.AluOpType.mult)
                v256 = stat.tile([128, SPT * 2], F32, tag="v256")
                nc.vector.tensor_tensor(out=v256[:], in0=m2e, in1=m2o, op=mybir.AluOpType.add)
                dm2s = stat.tile([128, SPT * 2], F32, tag="dm2s")
                nc.vector.tensor_scalar(
                    out=dm2s[:], in0=dm2[:], scalar1=64.0, scalar2=None,
                    op0=mybir.AluOpType.mult,
                )
                nc.vector.tensor_tensor(out=v256[:], in0=v256[:], in1=dm2s[:], op=mybir.AluOpType.add)
                # std = sqrt(v256/256 + eps); rstd = 1/std
                std = stat.tile([128, SPT * 2], F32, tag="std")
                nc.scalar.activation(
                    std[:], v256[:], mybir.ActivationFunctionType.Sqrt,
                    bias=eps_t[:], scale=1.0 / 256.0,
                )
                rstd = stat.tile([128, SPT * 2], F32, tag="rstd")
                nc.vector.reciprocal(rstd[:], std[:])
                # pack = (me + mo) * rstd  (= 2*mean*rstd; the 0.5 is folded into s1)
                msum = stat.tile([128, SPT * 2], F32, tag="msum")
                nc.vector.tensor_tensor(out=msum[:], in0=me, in1=mo, op=mybir.AluOpType.add)
                # pack in (br, sub) column order so the partition-major DMA
                # below lands rows as [br, sub*128+q]
                pack = stat.tile([128, 2, SPT], F32, tag="pack")
                nc.vector.tensor_tensor(
                    out=pack[:].rearrange("p b s -> p s b"), in0=msum[:], in1=rstd[:],
                    op=mybir.AluOpType.mult,
                )

                # pack -> row layout: PE transpose [128,8] -> psum [8,128] -> bf16 rows
                pk_ps = ps_r.tile([SPT * 2, 128], F32, tag="pk")
                nc.tensor.transpose(pk_ps[:], pack[:].rearrange("p b s -> p (b s)"), ident_f[:])
                pk_sb = rows.tile([SPT * 2, 128], BF16, tag="pk_sb")
                nc.scalar.copy(pk_sb[:], pk_ps[:])
                mrrow = rows.tile([1, 2, ST], BF16, tag="mrrow")
                # pk_sb rows are (br, sub); partition-major linearize
                nc.sync.dma_start(mrrow[:].rearrange("p b t -> p (b t)"), pk_sb[:])

                # diag tiles (gpsimd): diag(rstd) per (sub, br)
                diag_all = []
                for sub in range(SPT):
                    dg = diags.tile([128, 2, 128], BF16, tag="diag")
                    for br in range(2):
                        nc.gpsimd.tensor_scalar_mul(
                            dg[:, br, :], ident_bf[:], rstd[:, sub * 2 + br : sub * 2 + br + 1]
                        )
                    diag_all.append(dg)

                # diag-mm: zT[(br,c)][d, (sub tok)] = xb^T @ diag
                zt_sb = zts.tile([128, 2, 2, ST], BF16, tag="zt")
                for br in range(2):
                    for c in range(2):
                        zt_ps = ps_a.tile([128, ST], F32, tag="ps_a")
                        for sub in range(SPT):
                            nc.tensor.matmul(
                                zt_ps[:, sub * SUB : (sub + 1) * SUB],
                                xb_all[sub][:, br, c * 128 : (c + 1) * 128],
                                diag_all[sub][:, br, :],
                                start=True, stop=True,
                            )
                        nc.scalar.copy(zt_sb[:, br, c, :], zt_ps[:])

                # mm1 (+mean corr) + relu -> hs bf16
                hs_sb = hss.tile([128, 2, 2, ST], BF16, tag="hs")
                for br in range(2):
                    h_ps = ps_h.tile([128, 2, ST], F32, tag="h_ps")
                    for m in range(2):
                        for k in range(2):
                            nc.tensor.matmul(
                                h_ps[:, m, :],
                                w1_sb[:, l, br, k, m, :],
                                zt_sb[:, br, k, :],
                                start=(k == 0), stop=False,
                            )
                        nc.tensor.matmul(
                            h_ps[:, m, :],
                            s1_sb[:, l, br, m, :],
                            mrrow[:, br, :],
                            start=False, stop=True,
                        )
                        nc.scalar.activation(
                            hs_sb[:, br, m, :], h_ps[:, m, :],
                            mybir.ActivationFunctionType.Relu,
                            bias=b1_sb[:, l, br, m : m + 1] if has_b1 else zero_t[:],
                            scale=1.0,
                        )

                # mm2 (+b2) -> y psum [tok, (br,d)]; resid add -> x_new
                x_new_list = []
                for sub in range(SPT):
                    y_ps = ps_y.tile([128, 2, D], F32, tag="y_ps")
                    for br in range(2):
                        for m in range(2):
                            nc.tensor.matmul(
                                y_ps[:, br, :],
                                hs_sb[:, br, m, sub * SUB : (sub + 1) * SUB],
                                w2_sb[:, l, br, m, :],
                                start=(m == 0), stop=(m == 1) and not has_b2,
                            )
                        if has_b2:
                            nc.tensor.matmul(
                                y_ps[:, br, :], ones_sb[:], b2_sb[:, l, br, :],
                                start=False, stop=True,
                            )
                    x_new = xs.tile([128, 2, D], F32, tag="x")
                    nc.vector.tensor_tensor(
                        out=x_new[:], in0=x_cur[sub][:], in1=y_ps[:],
                        op=mybir.AluOpType.add,
                    )
                    x_new_list.append(x_new)
                    ts = t0 + sub * SUB
                    nc.sync.dma_start(
                        out_d[l, b, ts : ts + SUB, :],
                        x_new[:].rearrange("p b q -> p (b q)"),
                    )
                x_cur = x_new_list

    nc.compile()
    return nc


def _host_prep(inputs):
    x = np.asarray(inputs["inputs"], np.float32)
    lp = np.asarray(inputs["left_padding"], np.float32)
    rp = np.asarray(inputs["right_padding"], np.float32)
    lw = np.asarray(inputs["left_weights"], np.float32)
    rw = np.asarray(inputs["right_weights"], np.float32)

    xpad = np.concatenate(
        [np.broadcast_to(lp, (B, W, D)), x, np.broadcast_to(rp, (B, W, D))], axis=1
    ).astype(ml_dtypes.bfloat16)  # [B, S+2W, D]

    # band matrices: out_left[t] = sum_j lw[j] pad[t+j]; right shifted by W
    band = np.zeros((128 + 2 * W, 2, 128), np.float32)
    for i in range(128):
        band[i : i + W + 1, 0, i] = lw
        band[i + W : i + 2 * W + 1, 1, i] = rw
    banda = band[:128].reshape(128, 2 * 128).astype(ml_dtypes.bfloat16)
    bandb = band[128:].reshape(2 * W, 2 * 128).astype(ml_dtypes.bfloat16)

    # fold LN gamma/beta into W1/b1
    w1 = np.empty((L, 2, D, D), np.float32)
    b1 = np.empty((L, 2, D), np.float32)
    for bi, p in enumerate("lr"):
        pre = "left_" if p == "l" else "right_"
        g = np.asarray(inputs[pre + "ln_g"], np.float32)
        bb = np.asarray(inputs[pre + "ln_b"], np.float32)
        W1 = np.asarray(inputs[pre + "w1"], np.float32)
        B1 = np.asarray(inputs[pre + "b1"], np.float32)
        for li in range(L):
            w1[li, bi] = g[li][:, None] * W1[li]
            b1[li, bi] = B1[li] + bb[li] @ W1[li]
    w2 = np.stack(
        [np.asarray(inputs["left_w2"], np.float32), np.asarray(inputs["right_w2"], np.float32)], axis=1
    )
    b2 = np.stack(
        [np.asarray(inputs["left_b2"], np.float32), np.asarray(inputs["right_b2"], np.float32)], axis=1
    )

    # mean-correction lhsT rows: -0.5 * column sums of folded W1, computed on
    # the bf16-rounded weights to match what mm1 actually multiplies by
    w1_bf = w1.astype(ml_dtypes.bfloat16).astype(np.float32)
    s1 = -0.5 * w1_bf.sum(axis=2)  # [L, 2, D(out)]

    w1c = w1.reshape(L, 2, 2, 128, 2, 128).transpose(3, 0, 1, 2, 4, 5).reshape(128, -1)
    w2c = w2.reshape(L, 2, 2, 128, 256).transpose(3, 0, 1, 2, 4).reshape(128, -1)
    b1c = b1.reshape(L, 2, 2, 128).transpose(3, 0, 1, 2).reshape(128, -1)
    s1c = s1.reshape(L, 2, 2, 128).reshape(1, -1)

    shared = {
        "w1": np.ascontiguousarray(w1c.astype(ml_dtypes.bfloat16)),
        "w2": np.ascontiguousarray(w2c.astype(ml_dtypes.bfloat16)),
        "s1": np.ascontiguousarray(s1c.astype(ml_dtypes.bfloat16)),
        "b1": np.ascontiguousarray(b1c.astype(np.float32)),
        "b2": np.ascontiguousarray(b2.reshape(1, -1).astype(ml_dtypes.bfloat16)),
        "banda": np.ascontiguousarray(banda),
        "bandb": np.ascontiguousarray(bandb),
    }
    has_b1 = bool(np.any(b1 != 0))
    has_b2 = bool(np.any(b2 != 0))
    return xpad, shared, has_b1, has_b2


def kernel(**inputs):
    xpad, shared, has_b1, has_b2 = _host_prep(inputs)
    key = (has_b1, has_b2)
    if key not in _CACHE:
        _CACHE[key] = _build_program(has_b1, has_b2)
    nc = _CACHE[key]

    in_maps = [
        {"xpad": np.ascontiguousarray(xpad[c * BPC : (c + 1) * BPC]), **shared}
        for c in range(NCORES)
    ]
    res = run_bass_kernel_spmd(nc, in_maps, core_ids=list(range(NCORES)))

    all_layers = np.empty((L, B, S, 2 * D), np.float32)
    for c in range(NCORES):
        all_layers[:, c * BPC : (c + 1) * BPC] = res.results[c]["out_all"]
    return all_layers, all_layers[-1].copy()


# revision 9
# speedup vs baseline: 1.6375x; 1.6375x over previous
"""Trainium2 Bass kernel for nn_LBLResNetBiLm (v2: bf16 matmuls + diag-matmul LN apply).

Computation (see reference): pad sequence with learned boundary vectors,
take 9-tap left/right weighted window sums over the sequence, then run 3
residual pre-LN FFN layers per branch; emit every layer's concat(left,right).

Sharding: data-parallel over batch, 2 batch elements per core x 8 cores.

Per-core design (tokens on partitions, d on free dim; residual x in fp32):
- window conv = band-matrix matmul over host-built Toeplitz bands (bf16)
- LN stats: bn_stats per (subtile, branch); even/odd halves merged with
  batched [128,8] DVE ops; LN gamma/beta folded into W1/b1 on host
- LN apply + transpose fused into PE: zT = xb^T @ diag(rstd) (diag built
  by gpsimd from a const identity), mean correction folded into mm1 as a
  K=1 matmul with lhsT = -0.5*colsum(W1') and rhs = (mean_e+mean_o)*rstd
  row (moved to free-dim layout via a small PE transpose + sbuf DMA)
- mm1: W1 chunks stationary, N=512; relu+b1 on ACT -> hs bf16
- mm2: hs chunks stationary, W2 moving -> y psum lands in [token, d]
  layout; b2 via K=1 ones matmul only if b2 != 0
- resid: DVE tensor_tensor add (fp32, exact residual stream)
"""
import sys

sys.path.insert(0, "/opt/trn_rl_repo")

from contextlib import ExitStack

import ml_dtypes
import numpy as np

import concourse.bass as bass
import concourse.tile as tile
from concourse import bacc, mybir
from concourse.bass_utils import run_bass_kernel_spmd
from concourse.masks import make_identity

B, S, D, W, L = 16, 2048, 256, 8, 3
NCORES = 8
BPC = B // NCORES            # batches per core
SUB = 128                    # tokens per subtile (partition dim)
SPT = 4                      # subtiles per supertile
ST = SUB * SPT               # tokens per supertile
NST = BPC * S // ST          # supertiles per core
F32 = mybir.dt.float32
BF16 = mybir.dt.bfloat16
EPS = 1e-6

_CACHE = {}


def _build_program(has_b1, has_b2):
    nc = bacc.Bacc("TRN2", target_bir_lowering=False, debug=False, num_devices=NCORES)

    xpad_d = nc.dram_tensor("xpad", [BPC, S + 2 * W, D], BF16, kind="ExternalInput").ap()
    # w1: [128(din-in-chunk), (l br k m) * 128(dout-in-chunk)]
    w1_d = nc.dram_tensor("w1", [128, L * 8 * 128], BF16, kind="ExternalInput").ap()
    # w2: [128(dout-in-chunk), (l br m) * 256(d2)]
    w2_d = nc.dram_tensor("w2", [128, L * 4 * 256], BF16, kind="ExternalInput").ap()
    # s1: [1, (l br m) * 128] = -0.5 * colsum of folded W1
    s1_d = nc.dram_tensor("s1", [1, L * 4 * 128], BF16, kind="ExternalInput").ap()
    b1_d = nc.dram_tensor("b1", [128, L * 2 * 2], F32, kind="ExternalInput").ap()
    b2_d = nc.dram_tensor("b2", [1, L * 2 * 256], BF16, kind="ExternalInput").ap()
    banda_d = nc.dram_tensor("banda", [128, 2 * 128], BF16, kind="ExternalInput").ap()
    bandb_d = nc.dram_tensor("bandb", [2 * W, 2 * 128], BF16, kind="ExternalInput").ap()
    out_d = nc.dram_tensor("out_all", [L, BPC, S, 2 * D], F32, kind="ExternalOutput").ap()

    with tile.TileContext(nc) as tc, ExitStack() as ctx:
        const = ctx.enter_context(tc.tile_pool(name="const", bufs=1))
        pads = ctx.enter_context(tc.tile_pool(name="pads", bufs=4))
        xs = ctx.enter_context(tc.tile_pool(name="xs", bufs=16))
        xbs = ctx.enter_context(tc.tile_pool(name="xbs", bufs=10))
        diags = ctx.enter_context(tc.tile_pool(name="diags", bufs=10))
        zts = ctx.enter_context(tc.tile_pool(name="zts", bufs=3))
        hss = ctx.enter_context(tc.tile_pool(name="hss", bufs=3))
        stat = ctx.enter_context(tc.tile_pool(name="stat", bufs=3))
        rows = ctx.enter_context(tc.tile_pool(name="rows", bufs=4))
        ps_a = ctx.enter_context(tc.tile_pool(name="ps_a", bufs=2, space="PSUM"))
        ps_h = ctx.enter_context(tc.tile_pool(name="ps_h", bufs=1, space="PSUM"))
        ps_y = ctx.enter_context(tc.tile_pool(name="ps_y", bufs=3, space="PSUM"))
        ps_r = ctx.enter_context(tc.tile_pool(name="ps_r", bufs=1, space="PSUM"))

        # ---- constants
        w1_sb = const.tile([128, L, 2, 2, 2, 128], BF16)
        nc.sync.dma_start(w1_sb[:], w1_d[:].rearrange("p (l b k m q) -> p l b k m q", l=L, b=2, k=2, m=2))
        w2_sb = const.tile([128, L, 2, 2, 256], BF16)
        nc.sync.dma_start(w2_sb[:], w2_d[:].rearrange("p (l b m q) -> p l b m q", l=L, b=2, m=2))
        s1_sb = const.tile([1, L, 2, 2, 128], BF16)
        nc.sync.dma_start(s1_sb[:], s1_d[:].rearrange("p (l b m q) -> p l b m q", l=L, b=2, m=2))
        b1_sb = const.tile([128, L, 2, 2], F32)
        nc.sync.dma_start(b1_sb[:], b1_d[:].rearrange("p (l b m) -> p l b m", l=L, b=2))
        b2_sb = None
        ones_sb = None
        if has_b2:
            b2_sb = const.tile([1, L, 2, 256], BF16)
            nc.sync.dma_start(b2_sb[:], b2_d[:].rearrange("p (l b q) -> p l b q", l=L, b=2))
            ones_f = const.tile([1, 128], F32)
            nc.vector.memset(ones_f[:], 1.0)
            ones_sb = const.tile([1, 128], BF16)
            nc.vector.tensor_copy(ones_sb[:], ones_f[:])
        banda_sb = const.tile([128, 2, 128], BF16)
        nc.sync.dma_start(banda_sb[:], banda_d[:].rearrange("p (b q) -> p b q", b=2))
        bandb_sb = const.tile([2 * W, 2, 128], BF16)
        nc.sync.dma_start(bandb_sb[:], bandb_d[:].rearrange("p (b q) -> p b q", b=2))

        ident_f = const.tile([128, 128], F32)
        make_identity(nc, ident_f[:])
        ident_bf = const.tile([128, 128], BF16)
        nc.vector.tensor_copy(ident_bf[:], ident_f[:])
        eps_t = const.tile([128, 1], F32)
        nc.vector.memset(eps_t[:], EPS)
        zero_t = const.tile([128, 1], F32)
        nc.vector.memset(zero_t[:], 0.0)

        # ---- main loop
        for st in range(NST):
            b = (st * ST) // S
            t0 = (st * ST) % S

            # conv: x0[t, (br,d)] per subtile (bf16 matmul, fp32 psum -> fp32 x)
            x_cur = []
            for sub in range(SPT):
                ts = t0 + sub * SUB
                pad_a = pads.tile([128, D], BF16, tag="pad_a")
                nc.sync.dma_start(pad_a[:], xpad_d[b, ts : ts + 128, :])
                pad_b = pads.tile([2 * W, D], BF16, tag="pad_b")
                nc.sync.dma_start(pad_b[:], xpad_d[b, ts + 128 : ts + 128 + 2 * W, :])

                cps = ps_a.tile([128, 2 * D], F32, tag="ps_a")
                for br in range(2):
                    nc.tensor.matmul(
                        cps[:, br * D : (br + 1) * D],
                        banda_sb[:, br, :], pad_a[:], start=True, stop=False,
                    )
                    nc.tensor.matmul(
                        cps[:, br * D : (br + 1) * D],
                        bandb_sb[:, br, :], pad_b[:], start=False, stop=True,
                    )
                x_t = xs.tile([128, 2, D], F32, tag="x")
                nc.scalar.copy(x_t[:], cps[:].rearrange("p (b q) -> p b q", b=2))
                x_cur.append(x_t)

            for l in range(L):
                # xb cast (ACT) + bn stats (DVE)
                xb_all = []
                stats = stat.tile([128, SPT, 2, 6], F32, tag="bnst")
                for sub in range(SPT):
                    xb_t = xbs.tile([128, 2, D], BF16, tag="xb")
                    nc.scalar.copy(xb_t[:], x_cur[sub][:])
                    xb_all.append(xb_t)
                    for br in range(2):
                        nc.vector.bn_stats(stats[:, sub, br, :], xb_t[:, br, :])

                # batched stats finalize over [128, 8]  (cols = (sub, br))
                sv = stats[:].rearrange("p s b q -> p (s b) q")
                me, mo = sv[:, :, 1], sv[:, :, 4]
                m2e, m2o = sv[:, :, 2], sv[:, :, 5]
                dm = stat.tile([128, SPT * 2], F32, tag="dm")
                nc.vector.tensor_tensor(out=dm[:], in0=me, in1=mo, op=mybir.AluOpType.subtract)
                dm2 = stat.tile([128, SPT * 2], F32, tag="dm2")
                nc.vector.tensor_tensor(out=dm2[:], in0=dm[:], in1=dm[:], op=mybir.AluOpType.mult)
                v256 = stat.tile([128, SPT * 2], F32, tag="v256")
                nc.vector.tensor_tensor(out=v256[:], in0=m2e, in1=m2o, op=mybir.AluOpType.add)
                dm2s = stat.tile([128, SPT * 2], F32, tag="dm2s")
                nc.vector.tensor_scalar(
                    out=dm2s[:], in0=dm2[:], scalar1=64.0, scalar2=None,
                    op0=mybir.AluOpType.mult,
                )
                nc.vector.tensor_tensor(out=v256[:], in0=v256[:], in1=dm2s[:], op=mybir.AluOpType.add)
                # std = sqrt(v256/256 + eps); rstd = 1/std
                std = stat.tile([128, SPT * 2], F32, tag="std")
                nc.scalar.activation(
                    std[:], v256[:], mybir.ActivationFunctionType.Sqrt,
                    bias=eps_t[:], scale=1.0 / 256.0,
                )
                rstd = stat.tile([128, SPT * 2], F32, tag="rstd")
                nc.vector.reciprocal(rstd[:], std[:])
                # pack = (me + mo) * rstd  (= 2*mean*rstd; the 0.5 is folded into s1)
                msum = stat.tile([128, SPT * 2], F32, tag="msum")
                nc.vector.tensor_tensor(out=msum[:], in0=me, in1=mo, op=mybir.AluOpType.add)
                # pack in (br, sub) column order so the partition-major DMA
                # below lands rows as [br, sub*128+q]
                pack = stat.tile([128, 2, SPT], F32, tag="pack")
                nc.vector.tensor_tensor(
                    out=pack[:].rearrange("p b s -> p s b"), in0=msum[:], in1=rstd[:],
                    op=mybir.AluOpType.mult,
                )

                # pack -> row layout: PE transpose [128,8] -> psum [8,128] -> bf16 rows
                pk_ps = ps_r.tile([SPT * 2, 128], F32, tag="pk")
                nc.tensor.transpose(pk_ps[:], pack[:].rearrange("p b s -> p (b s)"), ident_f[:])
                pk_sb = rows.tile([SPT * 2, 128], BF16, tag="pk_sb")
                nc.scalar.copy(pk_sb[:], pk_ps[:])
                mrrow = rows.tile([1, 2, ST], BF16, tag="mrrow")
                # pk_sb rows are (br, sub); partition-major linearize
                nc.sync.dma_start(mrrow[:].rearrange("p b t -> p (b t)"), pk_sb[:])

                # diag tiles (gpsimd): diag(rstd) per (sub, br)
                diag_all = []
                for sub in range(SPT):
                    dg = diags.tile([128, 2, 128], BF16, tag="diag")
                    for br in range(2):
                        nc.vector.tensor_scalar_mul(
                            dg[:, br, :], ident_bf[:], rstd[:, sub * 2 + br : sub * 2 + br + 1]
                        )
                    diag_all.append(dg)

                # diag-mm: zT[(br,c)][d, (sub tok)] = xb^T @ diag
                zt_sb = zts.tile([128, 2, 2, ST], BF16, tag="zt")
                for br in range(2):
                    for c in range(2):
                        zt_ps = ps_a.tile([128, ST], F32, tag="ps_a")
                        for sub in range(SPT):
                            nc.tensor.matmul(
                                zt_ps[:, sub * SUB : (sub + 1) * SUB],
                                xb_all[sub][:, br, c * 128 : (c + 1) * 128],
                                diag_all[sub][:, br, :],
                                start=True, stop=True,
                            )
                        nc.scalar.copy(zt_sb[:, br, c, :], zt_ps[:])

                # mm1 (+mean corr) + relu -> hs bf16
                hs_sb = hss.tile([128, 2, 2, ST], BF16, tag="hs")
                for br in range(2):
                    h_ps = ps_h.tile([128, 2, ST], F32, tag="h_ps")
                    for m in range(2):
                        for k in range(2):
                            nc.tensor.matmul(
                                h_ps[:, m, :],
                                w1_sb[:, l, br, k, m, :],
                                zt_sb[:, br, k, :],
                                start=(k == 0), stop=False,
                            )
                        nc.tensor.matmul(
                            h_ps[:, m, :],
                            s1_sb[:, l, br, m, :],
                            mrrow[:, br, :],
                            start=False, stop=True,
                        )
                        nc.scalar.activation(
                            hs_sb[:, br, m, :], h_ps[:, m, :],
                            mybir.ActivationFunctionType.Relu,
                            bias=b1_sb[:, l, br, m : m + 1] if has_b1 else zero_t[:],
                            scale=1.0,
                        )

                # mm2 (+b2) -> y psum [tok, (br,d)]; resid add -> x_new
                x_new_list = []
                for sub in range(SPT):
                    y_ps = ps_y.tile([128, 2, D], F32, tag="y_ps")
                    for br in range(2):
                        for m in range(2):
                            nc.tensor.matmul(
                                y_ps[:, br, :],
                                hs_sb[:, br, m, sub * SUB : (sub + 1) * SUB],
                                w2_sb[:, l, br, m, :],
                                start=(m == 0), stop=(m == 1) and not has_b2,
                            )
                        if has_b2:
                            nc.tensor.matmul(
                                y_ps[:, br, :], ones_sb[:], b2_sb[:, l, br, :],
                                start=False, stop=True,
                            )
                    x_new = xs.tile([128, 2, D], F32, tag="x")
                    nc.vector.tensor_tensor(
                        out=x_new[:], in0=x_cur[sub][:], in1=y_ps[:],
                        op=mybir.AluOpType.add,
                    )
                    x_new_list.append(x_new)
                    ts = t0 + sub * SUB
                    nc.sync.dma_start(
                        out_d[l, b, ts : ts + SUB, :],
                        x_new[:].rearrange("p b q -> p (b q)"),
                    )
                x_cur = x_new_list

    nc.compile()
    return nc


def _host_prep(inputs):
    x = np.asarray(inputs["inputs"], np.float32)
    lp = np.asarray(inputs["left_padding"], np.float32)
    rp = np.asarray(inputs["right_padding"], np.float32)
    lw = np.asarray(inputs["left_weights"], np.float32)
    rw = np.asarray(inputs["right_weights"], np.float32)

    xpad = np.concatenate(
        [np.broadcast_to(lp, (B, W, D)), x, np.broadcast_to(rp, (B, W, D))], axis=1
    ).astype(ml_dtypes.bfloat16)  # [B, S+2W, D]

    # band matrices: out_left[t] = sum_j lw[j] pad[t+j]; right shifted by W
    band = np.zeros((128 + 2 * W, 2, 128), np.float32)
    for i in range(128):
        band[i : i + W + 1, 0, i] = lw
        band[i + W : i + 2 * W + 1, 1, i] = rw
    banda = band[:128].reshape(128, 2 * 128).astype(ml_dtypes.bfloat16)
    bandb = band[128:].reshape(2 * W, 2 * 128).astype(ml_dtypes.bfloat16)

    # fold LN gamma/beta into W1/b1
    w1 = np.empty((L, 2, D, D), np.float32)
    b1 = np.empty((L, 2, D), np.float32)
    for bi, p in enumerate("lr"):
        pre = "left_" if p == "l" else "right_"
        g = np.asarray(inputs[pre + "ln_g"], np.float32)
        bb = np.asarray(inputs[pre + "ln_b"], np.float32)
        W1 = np.asarray(inputs[pre + "w1"], np.float32)
        B1 = np.asarray(inputs[pre + "b1"], np.float32)
        for li in range(L):
            w1[li, bi] = g[li][:, None] * W1[li]
            b1[li, bi] = B1[li] + bb[li] @ W1[li]
    w2 = np.stack(
        [np.asarray(inputs["left_w2"], np.float32), np.asarray(inputs["right_w2"], np.float32)], axis=1
    )
    b2 = np.stack(
        [np.asarray(inputs["left_b2"], np.float32), np.asarray(inputs["right_b2"], np.float32)], axis=1
    )

    # mean-correction lhsT rows: -0.5 * column sums of folded W1, computed on
    # the bf16-rounded weights to match what mm1 actually multiplies by
    w1_bf = w1.astype(ml_dtypes.bfloat16).astype(np.float32)
    s1 = -0.5 * w1_bf.sum(axis=2)  # [L, 2, D(out)]

    w1c = w1.reshape(L, 2, 2, 128, 2, 128).transpose(3, 0, 1, 2, 4, 5).reshape(128, -1)
    w2c = w2.reshape(L, 2, 2, 128, 256).transpose(3, 0, 1, 2, 4).reshape(128, -1)
    b1c = b1.reshape(L, 2, 2, 128).transpose(3, 0, 1, 2).reshape(128, -1)
    s1c = s1.reshape(L, 2, 2, 128).reshape(1, -1)

    shared = {
        "w1": np.ascontiguousarray(w1c.astype(ml_dtypes.bfloat16)),
        "w2": np.ascontiguousarray(w2c.astype(ml_dtypes.bfloat16)),
        "s1": np.ascontiguousarray(s1c.astype(ml_dtypes.bfloat16)),
        "b1": np.ascontiguousarray(b1c.astype(np.float32)),
        "b2": np.ascontiguousarray(b2.reshape(1, -1).astype(ml_dtypes.bfloat16)),
        "banda": np.ascontiguousarray(banda),
        "bandb": np.ascontiguousarray(bandb),
    }
    has_b1 = bool(np.any(b1 != 0))
    has_b2 = bool(np.any(b2 != 0))
    return xpad, shared, has_b1, has_b2


def kernel(**inputs):
    xpad, shared, has_b1, has_b2 = _host_prep(inputs)
    key = (has_b1, has_b2)
    if key not in _CACHE:
        _CACHE[key] = _build_program(has_b1, has_b2)
    nc = _CACHE[key]

    in_maps = [
        {"xpad": np.ascontiguousarray(xpad[c * BPC : (c + 1) * BPC]), **shared}
        for c in range(NCORES)
    ]
    res = run_bass_kernel_spmd(nc, in_maps, core_ids=list(range(NCORES)))

    all_layers = np.empty((L, B, S, 2 * D), np.float32)
    for c in range(NCORES):
        all_layers[:, c * BPC : (c + 1) * BPC] = res.results[c]["out_all"]
    return all_layers, all_layers[-1].copy()
